# revision 6
# baseline (speedup 1.0000x reference)
"""Trainium2 Bass kernel for nn_LossEvaluator (sequential trading-loss scan).

Strategy: data-parallel over batch (256 -> 32 per core x 8 cores). The
512-step scan runs on-chip; each step's inputs are host-packed into one
contiguous block per step so the DMA stream is sequential. The inner
8-currency sequential loop is collapsed into a single linear-recurrence
tensor_tensor_scan along the free dimension.

Element layout per core (B=32 batch, NS=64 samples, C=8 currencies):
  partition p = (s % 4) * 32 + b   (128 partitions)
  f = s // 4                        (16)
  wide tiles:  (128, 16, 8)  [p][f][c]
  small tiles: (128, 16)     [p][f]
"""
import sys

sys.path.insert(0, "/opt/trn_rl_repo")

import numpy as np

T, NCUR, NS, BATCH = 512, 8, 64, 256
NCORES = 8
B = BATCH // NCORES          # 32 batch per core
F = NS // 4                  # 16
P = 128
LEV = 10.0

# column offsets inside the per-step input block
_WIDE = ["open_s", "close_s", "type_s", "frac", "fracacr",
         "open_lp", "close_lp", "type_lp"]
_NW = len(_WIDE) * 128           # 1024
_ZLP0 = _NW                      # 16 cols
_CMP0 = _NW + 16                 # 5 compact tiles of 8 cols
_NCOLS = _CMP0 + 5 * 8           # 1080


def _wideize(a):
    """(T, 8, 64, B) -> (T, 128, 16, 8)  [p][f][c]"""
    t = a.shape[0]
    a = a.reshape(t, NCUR, F, 4, B)
    return np.transpose(a, (0, 3, 4, 2, 1)).reshape(t, P, F, NCUR)


def _smallize(a):
    """(T, 64, B) -> (T, 128, 16)"""
    t = a.shape[0]
    a = a.reshape(t, F, 4, B)
    return np.transpose(a, (0, 2, 3, 1)).reshape(t, P, F)


def _compactize(a):
    """(T, 8, B) -> (T, 128, 8): tile[t, p, c] = a[t, c, p % 32]"""
    t = a.shape[0]
    out = np.transpose(a, (0, 2, 1))           # (T, B, 8)
    return np.tile(out, (1, 4, 1))             # (T, 128, 8)


def _unwideize(o):
    """(T, 128, 16, 8) -> (T, 8, 64, B)"""
    t = o.shape[0]
    o = o.reshape(t, 4, B, F, NCUR)
    return np.transpose(o, (0, 4, 3, 1, 2)).reshape(t, NCUR, NS, B)


def _unsmallize(o):
    """(T, 128, 16) -> (T, 64, B)"""
    t = o.shape[0]
    o = o.reshape(t, 4, B, F)
    return np.transpose(o, (0, 3, 1, 2)).reshape(t, NS, B)


def _install_drain_split():
    """walrus in this container rejects instructions carrying more than one
    sem wait; redistribute the Tile kernel-tail drain's waits over a chain
    of single-wait drains."""
    import bass_rust
    import concourse.tile as tile

    if getattr(tile.TileContext, "_drain_split_installed", False):
        return

    def _split_drain_and_barrier(self, tick_clock, wait_clock):
        nc = self.nc
        drain_inst = nc.sync.drain()
        wait_clock.add_sem_waits(
            drain_inst.ins, bass_rust.ScopedClock({None: tick_clock.global_clock})
        )
        si = drain_inst.ins.sync_info
        waits = list(si.on_wait) if si is not None else []
        if len(waits) > 1:
            drain_inst.ins.sync_info = bass_rust.SyncInfo(
                on_wait=waits[:1], on_update=list(si.on_update))
            for w in waits[1:]:
                extra = nc.sync.drain()
                extra.ins.sync_info = bass_rust.SyncInfo(on_wait=[w], on_update=[])

        nc.all_engine_barrier()
        assert self.sems is not None
        popped = nc._tile_sem_poison_stack.pop()
        assert popped is self._sem_poison
        nc.clear_and_free_semaphores(list(self.sems.allocated().values()))
        nc.all_engine_barrier()

    tile.TileContext._drain_and_barrier = _split_drain_and_barrier
    tile.TileContext._drain_split_installed = True


def build_nc(steps=T):
    """Build the Bass program (identical for all cores)."""
    import concourse.bacc as bacc
    import concourse.mybir as mybir
    import concourse.tile as tile

    _install_drain_split()

    f32 = mybir.dt.float32
    Alu = mybir.AluOpType
    Act = mybir.ActivationFunctionType

    nc = bacc.Bacc("TRN2", target_bir_lowering=False, debug=False)

    in_d = nc.dram_tensor("inblk", [steps, P, _NCOLS], f32, kind="ExternalInput")
    ps0_d = nc.dram_tensor("init_ps", [P, F, NCUR], f32, kind="ExternalInput")
    pt0_d = nc.dram_tensor("init_pt", [P, F, NCUR], f32, kind="ExternalInput")
    ops0_d = nc.dram_tensor("init_ops", [P, F, NCUR], f32, kind="ExternalInput")
    or0_d = nc.dram_tensor("init_or", [P, F, NCUR], f32, kind="ExternalInput")
    tm0_d = nc.dram_tensor("init_tm", [P, F], f32, kind="ExternalInput")
    g_d = nc.dram_tensor("cG", [P, 32], f32, kind="ExternalInput")
    h_d = nc.dram_tensor("cH", [32, P], f32, kind="ExternalInput")

    ps_o = nc.dram_tensor("out_ps", [steps, P, F, NCUR], f32, kind="ExternalOutput")
    pt_o = nc.dram_tensor("out_pt", [steps, P, F, NCUR], f32, kind="ExternalOutput")
    ops_o = nc.dram_tensor("out_ops", [steps, P, F, NCUR], f32, kind="ExternalOutput")
    or_o = nc.dram_tensor("out_or", [steps, P, F, NCUR], f32, kind="ExternalOutput")
    tm_o = nc.dram_tensor("out_tm", [steps, P, F], f32, kind="ExternalOutput")
    loss_o = nc.dram_tensor("out_loss", [P, F], f32, kind="ExternalOutput")

    WSH, SSH = [P, F, NCUR], [P, F]

    def wv(tile_, idx):
        """wide view: cols [idx*128, idx*128+128) as (P, F, C)"""
        return tile_[:, idx * 128:(idx + 1) * 128].rearrange(
            "p (f c) -> p f c", c=NCUR)

    with tile.TileContext(nc) as tc:
        with (
            tc.tile_pool(name="state", bufs=1) as st,
            tc.tile_pool(name="inp", bufs=3) as inp,
            tc.tile_pool(name="scr", bufs=2) as scr,
            tc.tile_pool(name="psum", bufs=2, space="PSUM") as psp,
        ):
            PS = st.tile(WSH, f32)
            PT = st.tile(WSH, f32)
            OPS = st.tile(WSH, f32)
            OR = st.tile(WSH, f32)
            OTL = st.tile(WSH, f32)
            OSL = st.tile(WSH, f32)
            CCL = st.tile(WSH, f32)
            TM = st.tile(SSH, f32)
            CZ = st.tile(SSH, f32)
            LOSS = st.tile(SSH, f32)
            SCE = st.tile(SSH, f32)
            ZW = st.tile(WSH, f32)
            CMASK = st.tile(WSH, f32)
            AD0 = st.tile(WSH, f32)
            AD1 = st.tile(WSH, f32)
            G = st.tile([P, 32], f32)
            H = st.tile([32, P], f32)

            nc.sync.dma_start(PS[:], ps0_d[:])
            nc.sync.dma_start(PT[:], pt0_d[:])
            nc.sync.dma_start(OPS[:], ops0_d[:])
            nc.sync.dma_start(OR[:], or0_d[:])
            nc.sync.dma_start(TM[:], tm0_d[:])
            nc.sync.dma_start(G[:], g_d[:])
            nc.sync.dma_start(H[:], h_d[:])
            nc.vector.memset(OTL[:], 0.0)
            nc.vector.memset(OSL[:], 0.0)
            nc.vector.memset(CCL[:], 0.0)
            nc.vector.memset(CZ[:], 0.0)
            nc.vector.memset(LOSS[:], 0.0)
            nc.vector.memset(SCE[:], 0.0)
            nc.vector.memset(ZW[:], 0.0)
            nc.gpsimd.memset(CMASK[:], 1.0)
            nc.gpsimd.memset(CMASK[:, :, 0:1], 0.0)
            nc.gpsimd.memset(AD0[:, :, 0:1], 0.0)

            for i in range(steps):
                IN = inp.tile([P, _NCOLS], f32)
                nc.sync.dma_start(IN[:], in_d[i])

                open_s = wv(IN, 0)
                close_s = wv(IN, 1)
                type_s = wv(IN, 2)
                frac = wv(IN, 3)
                fracacr = wv(IN, 4)
                open_lp = wv(IN, 5)
                close_lp = wv(IN, 6)
                type_lp = wv(IN, 7)
                zlp = IN[:, _ZLP0:_ZLP0 + 16]

                def cmp_view(j):
                    base = _CMP0 + j * 8
                    return IN[:, base:base + 8].unsqueeze(1).broadcast_to(WSH)

                r1c, drc, ir0c, dirc, iacrc = (cmp_view(j) for j in range(5))

                # ---- phase 1: open P/L (old state) ----
                acc = scr.tile(WSH, f32)
                nc.vector.tensor_mul(acc[:], OPS[:], iacrc)      # acc_open
                v1 = scr.tile(WSH, f32)
                nc.gpsimd.tensor_mul(v1[:], PT[:], dirc)
                v2 = scr.tile(WSH, f32)
                nc.gpsimd.tensor_tensor(v2[:], ir0c, v1[:], Alu.subtract)
                u = scr.tile(WSH, f32)
                nc.vector.scalar_tensor_tensor(u[:], v2[:], 1.0, PS[:],
                                               Alu.subtract, Alu.mult)
                h1 = scr.tile(WSH, f32)
                nc.vector.tensor_mul(h1[:], OR[:], u[:])
                ng = scr.tile(WSH, f32)
                nc.vector.scalar_tensor_tensor(ng[:], h1[:], -1.0, OR[:],
                                               Alu.mult, Alu.subtract)
                p1 = scr.tile(WSH, f32)                          # open_pl / 10
                nc.vector.scalar_tensor_tensor(p1[:], ng[:], 1.0, acc[:],
                                               Alu.add, Alu.mult)

                s1 = scr.tile(SSH, f32)
                nc.vector.tensor_reduce(s1[:].unsqueeze(2), p1[:],
                                        mybir.AxisListType.X, Alu.add)
                m_ = scr.tile(SSH, f32)
                nc.vector.tensor_reduce(m_[:].unsqueeze(2), acc[:],
                                        mybir.AxisListType.X, Alu.add)

                # closeout = (TM + 10*S1 - 0.5*M < 0)
                t_ = scr.tile(SSH, f32)
                nc.vector.scalar_tensor_tensor(t_[:], s1[:], LEV, TM[:],
                                               Alu.mult, Alu.add)
                w_ = scr.tile(SSH, f32)
                nc.vector.scalar_tensor_tensor(w_[:], m_[:], -0.5, t_[:],
                                               Alu.mult, Alu.add)
                co = scr.tile(SSH, f32)
                nc.vector.tensor_single_scalar(co[:], w_[:], 0.0, Alu.is_lt)
                ncm = scr.tile(SSH, f32)                         # 1 - closeout
                nc.scalar.activation(ncm[:], co[:], Act.Identity,
                                     bias=1.0, scale=-1.0)

                nc.gpsimd.tensor_add(CZ[:], CZ[:], zlp)

                ncm_b = ncm[:].unsqueeze(2).broadcast_to(WSH)
                co_b = co[:].unsqueeze(2).broadcast_to(WSH)

                # masks (old PS)
                z1n = scr.tile(WSH, f32)                         # -(1-PS)*ncm
                nc.vector.scalar_tensor_tensor(z1n[:], PS[:], 1.0, ncm_b,
                                               Alu.subtract, Alu.mult)
                om = scr.tile(WSH, f32)
                nc.vector.scalar_tensor_tensor(om[:], z1n[:], -1.0, open_s,
                                               Alu.mult, Alu.mult)
                z2 = scr.tile(WSH, f32)
                nc.gpsimd.tensor_tensor(z2[:], PS[:], ncm_b, Alu.mult)
                clm = scr.tile(WSH, f32)                         # masked close_lp
                nc.gpsimd.tensor_mul(clm[:], z2[:], close_lp)
                c1 = scr.tile(WSH, f32)
                nc.vector.tensor_tensor(c1[:], co_b, close_s, Alu.max)
                cm = scr.tile(WSH, f32)
                nc.gpsimd.tensor_mul(cm[:], PS[:], c1[:])
                olm = scr.tile(WSH, f32)                         # -masked open_lp
                nc.vector.tensor_mul(olm[:], z1n[:], open_lp)
                exc = scr.tile(WSH, f32)                         # exec_lp
                nc.gpsimd.tensor_sub(exc[:], clm[:], olm[:])

                exs = scr.tile(SSH, f32)
                nc.vector.tensor_reduce(exs[:].unsqueeze(2), exc[:],
                                        mybir.AxisListType.X, Alu.add)
                # A0 = TM - M into AD1 slot 0 (old TM)
                nc.gpsimd.tensor_sub(AD1[:, :, 0:1], TM[:].unsqueeze(2),
                                     m_[:].unsqueeze(2))
                # exec slot0 += SCE (old)  -> margin-avail scan input
                nc.gpsimd.tensor_add(exc[:, :, 0:1], exc[:, :, 0:1],
                                     SCE[:].unsqueeze(2))
                nc.vector.tensor_add(SCE[:], SCE[:], exs[:])

                # state flips
                sm = scr.tile(WSH, f32)
                nc.gpsimd.tensor_sub(sm[:], om[:], cm[:])
                nc.gpsimd.tensor_add(PS[:], PS[:], sm[:])
                nc.vector.copy_predicated(PT[:], om[:].bitcast(mybir.dt.uint32), type_s)
                n1 = scr.tile(WSH, f32)
                nc.gpsimd.tensor_tensor(n1[:], PT[:], drc, Alu.mult)
                n2 = scr.tile(WSH, f32)
                nc.gpsimd.tensor_tensor(n2[:], n1[:], r1c, Alu.add)
                nc.vector.copy_predicated(OR[:], om[:].bitcast(mybir.dt.uint32), n2[:])
                nc.vector.copy_predicated(OTL[:], om[:].bitcast(mybir.dt.uint32), type_lp)
                nc.gpsimd.tensor_add(CCL[:], CCL[:], clm[:])

                # costs + baseline
                costs = scr.tile(WSH, f32)                       # cm * open_pl/10
                nc.vector.tensor_mul(costs[:], cm[:], p1[:])
                pm1 = psp.tile([32, F * NCUR], f32)
                nc.tensor.matmul(pm1[:], G[:],
                                 costs[:].rearrange("p f c -> p (f c)"))
                bb = scr.tile([32, NCUR], f32)
                nc.vector.tensor_reduce(
                    bb[:].unsqueeze(2),
                    pm1[:].rearrange("b (f c) -> b f c", c=NCUR).transpose([0, 2, 1]),
                    mybir.AxisListType.X, Alu.add)
                pm2 = psp.tile([P, NCUR], f32)
                nc.tensor.matmul(pm2[:], H[:], bb[:])

                # A-scan inputs
                g3 = scr.tile(WSH, f32)
                nc.vector.tensor_mul(g3[:], om[:], frac)
                nc.scalar.activation(AD0[:, :, 1:NCUR], g3[:, :, 0:NCUR - 1],
                                     Act.Identity, bias=1.0, scale=-1.0)
                sm2 = scr.tile(WSH, f32)
                nc.gpsimd.tensor_add(sm2[:], om[:], cm[:])
                b1 = scr.tile(WSH, f32)
                nc.gpsimd.tensor_mul(b1[:], acc[:], sm2[:])
                nc.vector.scalar_tensor_tensor(
                    AD1[:, :, 1:NCUR], costs[:, :, 0:NCUR - 1], LEV,
                    b1[:, :, 0:NCUR - 1], Alu.mult, Alu.add)
                aw = scr.tile(WSH, f32)
                nc.vector.tensor_tensor_scan(
                    aw[:].rearrange("p f c -> p (f c)"),
                    AD0[:].rearrange("p f c -> p (f c)"),
                    AD1[:].rearrange("p f c -> p (f c)"),
                    0.0, Alu.mult, Alu.add)
                mav = scr.tile(WSH, f32)
                nc.vector.tensor_tensor_scan(
                    mav[:].rearrange("p f c -> p (f c)"),
                    CMASK[:].rearrange("p f c -> p (f c)"),
                    exc[:].rearrange("p f c -> p (f c)"),
                    0.0, Alu.mult, Alu.add)
                nsw = scr.tile(WSH, f32)
                nc.vector.tensor_mul(nsw[:], fracacr, aw[:])
                nc.vector.copy_predicated(OPS[:], om[:].bitcast(mybir.dt.uint32), nsw[:])
                nc.vector.copy_predicated(OPS[:], cm[:].bitcast(mybir.dt.uint32), ZW[:])
                nc.vector.copy_predicated(OSL[:], om[:].bitcast(mybir.dt.uint32), mav[:])

                # loss
                clpa = scr.tile(WSH, f32)
                nc.gpsimd.tensor_add(clpa[:], OSL[:], OTL[:])
                clpb = scr.tile(WSH, f32)
                nc.gpsimd.tensor_add(clpb[:], clpa[:], CCL[:])
                clp = scr.tile(WSH, f32)
                nc.gpsimd.tensor_tensor(clp[:], clpb[:],
                                        CZ[:].unsqueeze(2).broadcast_to(WSH),
                                        Alu.add)
                cd = scr.tile(WSH, f32)
                nc.vector.tensor_sub(cd[:], costs[:],
                                     pm2[:].unsqueeze(1).broadcast_to(WSH))
                lt = scr.tile(WSH, f32)
                nc.vector.tensor_mul(lt[:], clp[:], cd[:])
                lt2 = scr.tile(WSH, f32)
                nc.vector.tensor_add(lt2[:], lt[:], costs[:])
                ltm = scr.tile(WSH, f32)
                nc.vector.tensor_mul(ltm[:], cm[:], lt2[:])
                ls = scr.tile(SSH, f32)
                nc.vector.tensor_reduce(ls[:].unsqueeze(2), ltm[:],
                                        mybir.AxisListType.X, Alu.add)
                nc.vector.scalar_tensor_tensor(LOSS[:], ls[:], LEV, LOSS[:],
                                               Alu.mult, Alu.add)
                tms = scr.tile(SSH, f32)
                nc.vector.tensor_reduce(tms[:].unsqueeze(2), costs[:],
                                        mybir.AxisListType.X, Alu.add)
                nc.vector.scalar_tensor_tensor(TM[:], tms[:], LEV, TM[:],
                                               Alu.mult, Alu.add)
                nc.vector.copy_predicated(CCL[:], cm[:].bitcast(mybir.dt.uint32), ZW[:])

                # step outputs
                nc.scalar.dma_start(ps_o[i], PS[:])
                nc.scalar.dma_start(pt_o[i], PT[:])
                nc.gpsimd.dma_start(ops_o[i], OPS[:])
                nc.sync.dma_start(or_o[i], OR[:])
                nc.sync.dma_start(tm_o[i], TM[:])

            nc.sync.dma_start(loss_o[:], LOSS[:])

    nc.compile()
    return nc


def _pack_core(inputs, k, steps=T):
    """Build per-core input map for core k."""
    f32 = np.float32
    sl = slice(k * B, (k + 1) * B)
    smp = np.asarray(inputs["samples"], f32)[:steps, :, :, :, sl]
    frc = np.asarray(inputs["fractions"], f32)[:steps, :, :, sl]
    xlp = np.asarray(inputs["x_logprobs"], f32)[:steps, :, :, :, sl]
    zlp = np.asarray(inputs["z_logprobs"], f32)[:steps, :, sl]
    rts = np.asarray(inputs["rates"], f32)[:steps, :, :, sl]
    acr = np.asarray(inputs["account_cur_rates"], f32)[:steps, :, sl]

    blk = np.empty((steps, P, _NCOLS), f32)
    wide_srcs = [smp[:, 0], smp[:, 1], smp[:, 2], frc,
                 frc * acr[:, :, None, :],
                 xlp[:, 0], xlp[:, 1], xlp[:, 2]]
    for idx, srcv in enumerate(wide_srcs):
        blk[:, :, idx * 128:(idx + 1) * 128] = _wideize(srcv).reshape(steps, P, 128)
    blk[:, :, _ZLP0:_ZLP0 + 16] = _smallize(zlp)
    r0, r1 = rts[:, 0], rts[:, 1]
    comp = [r1, r0 - r1, 1.0 / r0, 1.0 / r0 - 1.0 / r1, 1.0 / acr]
    for j, cv in enumerate(comp):
        base = _CMP0 + j * 8
        blk[:, :, base:base + 8] = _compactize(cv.astype(f32))

    gv = np.zeros((P, 32), f32)
    gv[np.arange(P), np.arange(P) % 32] = 1.0 / NS
    hv = np.zeros((32, P), f32)
    hv[np.arange(P) % 32, np.arange(P)] = 1.0

    def w1(x):
        return _wideize(x[None])[0]

    return {
        "inblk": blk,
        "init_ps": w1(np.asarray(inputs["pos_states"])[:, :, sl].astype(f32)),
        "init_pt": w1(np.asarray(inputs["pos_types"])[:, :, sl].astype(f32)),
        "init_ops": w1(np.asarray(inputs["open_pos_sizes"], f32)[:, :, sl]),
        "init_or": w1(np.asarray(inputs["open_rates"], f32)[:, :, sl]),
        "init_tm": _smallize(np.asarray(inputs["total_margin"], f32)[None, :, sl])[0],
        "cG": gv,
        "cH": hv,
    }


def _unpack(results, steps=T):
    """Assemble full outputs from per-core result dicts."""
    f32 = np.float32
    loss = np.concatenate(
        [_unsmallize(r["out_loss"][None])[0] for r in results], axis=-1)
    ps = np.concatenate([_unwideize(r["out_ps"]) for r in results], axis=-1)
    pt = np.concatenate([_unwideize(r["out_pt"]) for r in results], axis=-1)
    ops = np.concatenate([_unwideize(r["out_ops"]) for r in results], axis=-1)
    or_ = np.concatenate([_unwideize(r["out_or"]) for r in results], axis=-1)
    tm = np.concatenate([_unsmallize(r["out_tm"]) for r in results], axis=-1)
    return (loss.astype(f32),
            np.rint(ps).astype(np.int32),
            np.rint(pt).astype(np.int32),
            tm.astype(f32), ops.astype(f32), or_.astype(f32))


_NC_CACHE = {}


def kernel(samples, fractions, x_logprobs, z_logprobs, rates, account_cur_rates,
           pos_states, pos_types, total_margin, open_pos_sizes, open_rates):
    from concourse.bass_utils import run_bass_kernel_spmd

    inputs = dict(samples=samples, fractions=fractions, x_logprobs=x_logprobs,
                  z_logprobs=z_logprobs, rates=rates,
                  account_cur_rates=account_cur_rates, pos_states=pos_states,
                  pos_types=pos_types, total_margin=total_margin,
                  open_pos_sizes=open_pos_sizes, open_rates=open_rates)
    if T not in _NC_CACHE:
        _NC_CACHE[T] = build_nc(T)
    nc = _NC_CACHE[T]
    in_maps = [_pack_core(inputs, k) for k in range(NCORES)]
    res = run_bass_kernel_spmd(nc, in_maps, list(range(NCORES)))
    return _unpack(res.results)
